# revision 2
# baseline (speedup 1.0000x reference)
"""CoxPH loss kernel v4 for Trainium2, sharded across 8 NeuronCores.

Row-shard the n dimension (core c owns i in [c*2048, (c+1)*2048)); the
risk-set row-block R[i, j] is built on the fly in SBUF as fp8 mask tiles and
reduced over j by the tensor engine (bf16, 128-j k-tile per matmul; the PE
streams 128 mask elements/cycle, which is the binding resource).

Masks come from two engines in parallel (measured rates per [128,2048] op):
  - DVE, chunks 0..91: tensor_scalar with a uint16 tensor input runs in the
    DVE's 2x perf mode (~800ns vs 1336ns for f32). Compare only the high 16
    bits A = floor(u/256) of u = time*2^24: (A_i is_le A_j) * e_j -> bf16.
    A-collisions (pairs equal in the top 16 bits but not all 24) mis-rank
    ~2k of the 268M pairs; measured loss error ~6e-5, far below the 2e-2
    gate. e_j rides the second scalar slot (f32, full precision) and the
    product quantizes to fp8e4 on write (~3% rounding, random sign).
  - ACT, chunks 92..127: sigmoid(64*(u_j + 1) - 64*u_i) = exact 0/1 on the
    even-int grid; e_j goes into the fp8 matmul weights instead.

DoubleRow weights w8[p, r, dc] (k-tile pair at stride 128 columns, as the
dual-fp8 ldweights ISA restriction requires): 1.0 for DVE double-chunks,
fp8(e_j) for ACT double-chunks. DVE and ACT double-chunks are interleaved in
issue order so both engines stay busy.

Tail identical to v1/v2. Host-side prep is layout only: u = time*2^24,
A = u>>8 (uint16 + f32 copies), broadcasts, and slicing.
"""
import sys

sys.path.insert(0, "/opt/trn_rl_repo")

import numpy as np
import ml_dtypes
from contextlib import ExitStack

import concourse.bass as bass
import concourse.tile as tile
from concourse import mybir
from concourse import bass_utils

N = 16384
NCORES = 8
NLOC = N // NCORES  # 2048 rows per core
P = 128
JCH = N // P  # 128 j-chunks
NDC = JCH // 2  # 64 double-chunks
IB = 512  # psum bank width (f32)
NB = NLOC // IB  # 4 psum banks

F32 = mybir.dt.float32
BF16 = mybir.dt.bfloat16
FP16 = mybir.dt.float16
U16 = mybir.dt.uint16
FP8 = mybir.dt.float8e4
I32 = mybir.dt.int32

USCALE = float(2**24)
SIGS = 64.0
NDC_DVE = 46  # double-chunks 0..45 -> DVE (u16 2x compare), 46..63 -> ACT



# --------------------------------------------------------------------------
# post-trace IR fixups (same as v1)
# --------------------------------------------------------------------------
def _split_sync_waits(nc, max_waits=1):
    """Walrus's TPB_CTRL NO_STRUCT encoding rejects >1 sync wait per
    instruction (and drains use their wait slots internally); move excess
    waits onto preceding NOPs."""
    for f in nc.m.functions:
        for bb in f.blocks:
            new_insts = []
            for ins in bb.instructions:
                si = getattr(ins, "sync_info", None)
                cap = 0 if type(ins).__name__ == "InstDrain" else max_waits
                if si is not None and si.on_wait and len(si.on_wait) > cap:
                    waits = list(si.on_wait)
                    extra = waits if cap == 0 else waits[:-cap]
                    keep = [] if cap == 0 else waits[-cap:]
                    for i in range(0, len(extra), max_waits):
                        nop = mybir.InstNoOp(
                            name=nc.get_next_instruction_name(),
                            sync_info=mybir.SyncInfo(
                                on_wait=extra[i : i + max_waits], on_update=[]
                            ),
                            bass_nofuse=True,
                            engine=ins.engine,
                        )
                        new_insts.append(nop)
                    si.on_wait = keep
                new_insts.append(ins)
            bb.instructions[:] = new_insts


def _hoist_startup(nc):
    """Move wait-free input DMA triggers (and the dummy activation that
    carries walrus's ~1.3us PWP table load) ahead of the preamble barrier
    so they overlap engine synchronization."""
    f = nc.m.functions[0]
    main_bb, body_bb = f.blocks[0], f.blocks[1]
    moved = {mybir.EngineType.SP: [], mybir.EngineType.Activation: []}
    kept = []
    for pos, ins in enumerate(body_bb.instructions):
        si = getattr(ins, "sync_info", None)
        ty = type(ins).__name__
        hoistable = ty == "InstDMACopy" or ty == "InstActivation"
        if (
            pos < 25
            and hoistable
            and ins.engine in moved
            and not (si and si.on_wait)
        ):
            moved[ins.engine].append(ins)
        else:
            kept.append(ins)
    body_bb.instructions[:] = kept
    for eng, insts in moved.items():
        idx = next(
            i
            for i, ins in enumerate(main_bb.instructions)
            if type(ins).__name__ == "InstDrain" and ins.engine == eng
        )
        main_bb.instructions[idx:idx] = insts


def _trim_exit(nc):
    """The Tile exit sequence (two EVSEM barrier rounds + semaphore clear)
    costs ~9us. Only the SP drain matters at NEFF end: it holds the SP
    sequencer until the output DMAs complete. Keep just that."""
    f = nc.m.functions[0]
    end_bb = f.blocks[-1]
    keep = next(
        ins
        for ins in end_bb.instructions
        if type(ins).__name__ == "InstDrain" and ins.engine == mybir.EngineType.SP
    )
    end_bb.instructions[:] = [keep]


# --------------------------------------------------------------------------
# program build
# --------------------------------------------------------------------------
_prog_cache = {}


def _build_program(variant="fast", fixups=True):
    """variant "fast": u16 2x A-compare on DVE + sigmoid on ACT (needs the
    even-int u grid). variant "exact": f32 u-compare on DVE for all chunks
    (exact for arbitrary f32 times, ~1.7x slower)."""
    key = (variant, fixups)
    if key in _prog_cache:
        return _prog_cache[key]

    ndc_dve = NDC_DVE if variant == "fast" else NDC

    nc = bass.Bass("TRN2", target_bir_lowering=False, debug=False)

    t_full = nc.dram_tensor("t_full", [N], F32, kind="ExternalInput").ap()
    r_full = nc.dram_tensor("r_full", [N], F32, kind="ExternalInput").ap()
    a_in = nc.dram_tensor("a_loc", [P, NLOC], U16, kind="ExternalInput").ap()
    af_full = nc.dram_tensor("af_full", [N], F32, kind="ExternalInput").ap()
    ulb_in = nc.dram_tensor("ulb", [P, NLOC], F32, kind="ExternalInput").ap()
    r_loc = nc.dram_tensor("r_loc", [NLOC], F32, kind="ExternalInput").ap()
    ev_loc = nc.dram_tensor("ev_loc", [NLOC], I32, kind="ExternalInput").ap()
    num_out = nc.dram_tensor("num", [1, 1], F32, kind="ExternalOutput").ap()
    evs_out = nc.dram_tensor("evs", [1, 1], F32, kind="ExternalOutput").ap()

    def pcol(ap_1d, n_free):
        # [n] dram -> [128, n_free] row-major: out[p, c] = in[p*n_free + c]
        return bass.AP(
            tensor=ap_1d.tensor, offset=ap_1d.offset, ap=[[n_free, P], [1, n_free]]
        )

    def row(ap_1d, n_free):
        return bass.AP(
            tensor=ap_1d.tensor, offset=ap_1d.offset, ap=[[0, 1], [1, n_free]]
        )

    with tile.TileContext(nc) as tc, ExitStack() as ctx:
        singles = ctx.enter_context(tc.tile_pool(name="singles", bufs=1))
        mask_pool = ctx.enter_context(tc.tile_pool(name="mask", bufs=4))
        psum_pool = ctx.enter_context(tc.tile_pool(name="psum", bufs=1, space="PSUM"))

        # dummy activation carrying the PWP table load (hoisted pre-barrier)
        dummy_sb = singles.tile([1, 1], F32)
        nc.scalar.activation(
            dummy_sb[:],
            nc.const_aps.tensor(0.0, (1, 1)),
            mybir.ActivationFunctionType.Sigmoid,
        )

        # --- stage 0: loads + prep ------------------------------------
        a_sb = singles.tile([P, NLOC], U16)
        nc.sync.dma_start(a_sb[:], a_in[:])
        af_sb = singles.tile([P, JCH], F32)
        nc.sync.dma_start(af_sb[:], pcol(af_full, JCH))
        t_sb = singles.tile([P, JCH], F32)
        nc.sync.dma_start(t_sb[:], pcol(t_full, JCH))
        r_sb = singles.tile([P, JCH], F32)
        nc.scalar.dma_start(r_sb[:], pcol(r_full, JCH))
        ulb_sb = singles.tile([P, NLOC], F32)
        nc.sync.dma_start(ulb_sb[:], ulb_in[:])

        th_sb = singles.tile([P, JCH], F32)
        nc.vector.tensor_scalar(
            th_sb[:], r_sb[:], -20.0, 20.0, mybir.AluOpType.max, mybir.AluOpType.min
        )
        e_sb = singles.tile([P, JCH], F32)
        nc.scalar.activation(e_sb[:], th_sb[:], mybir.ActivationFunctionType.Exp)

        # fp8 weights, dual-row layout (k-tile pair at stride 128 columns --
        # the dual-fp8 ldweights ISA restriction): 1.0 for DVE double-chunks
        # (e is folded into the mask there), fp8(e_j) for ACT double-chunks:
        # e8w[p, r, dc] = e[j = p*128 + 2*dc + r] for dc >= NDC_DVE.
        e8w = singles.tile([P, 2, 128], BF16)
        nc.vector.memset(e8w[:, :, 0:ndc_dve], 1.0)
        if ndc_dve < NDC:
            e_base = e_sb[:]
            nc.vector.tensor_copy(
                e8w[:, :, ndc_dve:NDC],
                bass.AP(
                    tensor=e_base.tensor,
                    offset=e_base.offset + 2 * ndc_dve,
                    ap=[e_base.ap[0], [1, 2], [2, NDC - ndc_dve]],
                ),
            )
        if variant == "exact":
            u_sb = singles.tile([P, JCH], F32)
            nc.vector.tensor_scalar_mul(u_sb[:], t_sb[:], USCALE)

        # sigmoid bias 64*(u_j + 1)
        sigb_sb = singles.tile([P, JCH], F32)
        nc.vector.tensor_scalar(
            sigb_sb[:],
            t_sb[:],
            USCALE * SIGS,
            SIGS,
            mybir.AluOpType.mult,
            mybir.AluOpType.add,
        )

        # --- stage 1: masked accumulation over all j ------------------
        psums = [psum_pool.tile([1, IB], F32, name=f"psum{i}") for i in range(NB)]
        # interleave DVE and ACT double-chunks in issue order (Bresenham
        # merge) so both generator engines stay busy throughout
        dve_dcs = list(range(ndc_dve))
        act_dcs = list(range(ndc_dve, NDC))
        order = []
        acc = 0
        while dve_dcs or act_dcs:
            acc += len(act_dcs) and (NDC - ndc_dve)
            if dve_dcs:
                order.append(dve_dcs.pop(0))
            if acc >= ndc_dve or not dve_dcs:
                if act_dcs:
                    order.append(act_dcs.pop(0))
                acc -= ndc_dve
        for k, dc in enumerate(order):
            me8 = mask_pool.tile([P, 2, NLOC], BF16)
            for r in range(2):
                c = 2 * dc + r
                if dc >= ndc_dve:
                    nc.scalar.activation(
                        me8[:, r, :],
                        ulb_sb[:],
                        mybir.ActivationFunctionType.Sigmoid,
                        bias=sigb_sb[:, c : c + 1],
                        scale=-SIGS,
                    )
                elif variant == "fast":
                    nc.vector.tensor_scalar(
                        me8[:, r, :],
                        a_sb[:],
                        af_sb[:, c : c + 1],
                        e_sb[:, c : c + 1],
                        mybir.AluOpType.is_le,
                        mybir.AluOpType.mult,
                    )
                else:
                    nc.vector.tensor_scalar(
                        me8[:, r, :],
                        ulb_sb[:],
                        u_sb[:, c : c + 1],
                        e_sb[:, c : c + 1],
                        mybir.AluOpType.is_le,
                        mybir.AluOpType.mult,
                    )
            for r in range(2):
                for ib in range(NB):
                    nc.tensor.matmul(
                        psums[ib][:],
                        lhsT=e8w[:, r, dc : dc + 1],
                        rhs=me8[:, r, ib * IB : (ib + 1) * IB],
                        start=(k == 0 and r == 0),
                        stop=(k == NDC - 1 and r == 1),
                    )

        # --- numerator prep (overlaps the loop) -----------------------
        rloc_row = singles.tile([1, NLOC], F32)
        nc.scalar.dma_start(rloc_row[:], row(r_loc, NLOC))
        theta_row = singles.tile([1, NLOC], F32)
        nc.vector.tensor_scalar(
            theta_row[:],
            rloc_row[:],
            -20.0,
            20.0,
            mybir.AluOpType.max,
            mybir.AluOpType.min,
        )

        evi_row = singles.tile([1, NLOC], I32)
        nc.sync.dma_start(evi_row[:], row(ev_loc, NLOC))
        evf_row = singles.tile([1, NLOC], F32)
        nc.vector.tensor_copy(evf_row[:], evi_row[:])

        thev_row = singles.tile([1, NLOC], F32)
        s1_sb = singles.tile([1, 1], F32)
        nc.vector.scalar_tensor_tensor(
            thev_row[:],
            theta_row[:],
            1.0,
            evf_row[:],
            mybir.AluOpType.mult,
            mybir.AluOpType.mult,
            accum_out=s1_sb[:],
        )
        evcopy_row = singles.tile([1, NLOC], F32)
        evs_sb = singles.tile([1, 1], F32)
        nc.vector.tensor_scalar(
            evcopy_row[:],
            evf_row[:],
            0.0,
            None,
            mybir.AluOpType.add,
            mybir.AluOpType.add,
            accum_out=evs_sb[:],
        )

        # --- stage 2: log-denominator + partial loss ------------------
        one_sb = singles.tile([1, 1], F32)
        nc.vector.memset(one_sb[:], 1.0)
        y_row = singles.tile([1, NLOC], F32)
        ls_sb = singles.tile([1, NB], F32)
        for ib in range(NB):
            nc.vector.scalar_tensor_tensor(
                y_row[:, ib * IB : (ib + 1) * IB],
                psums[ib][:],
                1.0,
                evf_row[:, ib * IB : (ib + 1) * IB],
                mybir.AluOpType.subtract,
                mybir.AluOpType.mult,
            )
            nc.scalar.activation(
                y_row[:, ib * IB : (ib + 1) * IB],
                y_row[:, ib * IB : (ib + 1) * IB],
                mybir.ActivationFunctionType.Ln,
                bias=one_sb[:],
                accum_out=ls_sb[:, ib : ib + 1],
            )

        sumlog_sb = singles.tile([1, 1], F32)
        nc.vector.tensor_reduce(
            sumlog_sb[:], ls_sb[:], mybir.AxisListType.X, mybir.AluOpType.add
        )
        num_sb = singles.tile([1, 1], F32)
        nc.vector.tensor_tensor(
            num_sb[:], s1_sb[:], sumlog_sb[:], mybir.AluOpType.subtract
        )

        nc.sync.dma_start(num_out[:], num_sb[:])
        nc.sync.dma_start(evs_out[:], evs_sb[:])

    if fixups:
        _hoist_startup(nc)
        _trim_exit(nc)
        _split_sync_waits(nc)
    _prog_cache[key] = nc
    return nc


def _grid_ok(time):
    """True if time * 2^24 are all even integers < 2^24 (the jax uniform
    f32 grid), which both mask paths rely on."""
    u = np.asarray(time, np.float64) * USCALE
    if u.size == 0:
        return True
    return bool(
        np.all(u == np.round(u))
        and np.all(u % 2 == 0)
        and u.max() + 1 < USCALE
        and u.min() >= 0
    )


def _make_in_maps(risk, time, event):
    u = np.asarray(time, np.float64) * USCALE
    A = np.floor(u / 256.0)
    A16 = A.astype(np.uint16)
    Af = A.astype(np.float32)
    uf = u.astype(np.float32)
    in_maps = []
    for c in range(NCORES):
        s = slice(c * NLOC, (c + 1) * NLOC)
        in_maps.append(
            {
                "t_full": time,
                "r_full": risk,
                "af_full": Af,
                "a_loc": np.ascontiguousarray(np.broadcast_to(A16[s], (P, NLOC))),
                "ulb": np.ascontiguousarray(np.broadcast_to(uf[s], (P, NLOC))),
                "r_loc": risk[s].copy(),
                "ev_loc": event[s].copy(),
            }
        )
    return in_maps


def _run(risk, time, event, trace=False, tmpdir=None):
    nc = _build_program("fast" if _grid_ok(time) else "exact")
    return bass_utils.run_bass_kernel_spmd(
        nc,
        _make_in_maps(risk, time, event),
        core_ids=list(range(NCORES)),
        trace=trace,
        tmpdir=tmpdir,
    )


def kernel(risk, time, event):
    risk = np.ascontiguousarray(np.asarray(risk, dtype=np.float32))
    time = np.ascontiguousarray(np.asarray(time, dtype=np.float32))
    event = np.ascontiguousarray(np.asarray(event, dtype=np.int32))

    res = _run(risk, time, event)

    num = sum(float(res.results[c]["num"][0, 0]) for c in range(NCORES))
    evs = sum(float(res.results[c]["evs"][0, 0]) for c in range(NCORES))
    return np.float32(-(num / (evs + 1e-8)))


def profile(np_inputs, tmpdir=None):
    risk = np.ascontiguousarray(np.asarray(np_inputs["risk"], dtype=np.float32))
    time = np.ascontiguousarray(np.asarray(np_inputs["time"], dtype=np.float32))
    event = np.ascontiguousarray(np.asarray(np_inputs["event"], dtype=np.int32))
    res = _run(risk, time, event, trace=True, tmpdir=tmpdir)
    if res.instructions_and_trace is not None:
        print("trace:", res.instructions_and_trace[1])
    print(
        "mean_exec_time_ns:",
        res.mean_exec_time_ns,
        "max core:",
        res.max_exec_time_core_id,
    )
    return res.exec_time_ns
